# revision 11
# baseline (speedup 1.0000x reference)
"""Trainium2 Bass kernel for a Mamba block (embed lookup -> residual add ->
RMSNorm -> Mamba(in_proj, causal conv1d, selective scan, out_proj)).

v2 design:
- preamble (embed gather + residual + RMSNorm) token-sharded; hs kept in SBUF
  d-major (PE transposes), no AllGather.
- in_proj computed token-major with the FULL weight streamed from HBM; the
  xz activations are resharded to d-major via AllToAll (x and z separately).
- causal depthwise conv1d on the PE (per-channel diag-weight matmuls).
- x_proj partials AllReduced per batch entry.
- selective scan: chained tensor_tensor_scan over [8 states x (LC+1)] with
  carry-injection columns; dA via per-state Exp with per-partition A scale;
  softplus/silu as single fused activations; bf16 tree reduce over states.
- out_proj partials DMAd straight from PSUM to HBM; ReduceScatter per chunk.
"""

import numpy as np
import ml_dtypes

import concourse.bacc as bacc
import concourse.bass as bass
import concourse.mybir as mybir
import concourse.tile as tile
from concourse.bass import AP, IndirectOffsetOnAxis
from concourse.bass_utils import run_bass_kernel_spmd
from concourse.masks import make_identity

F32 = mybir.dt.float32
BF16 = mybir.dt.bfloat16
I32 = mybir.dt.int32
AF = mybir.ActivationFunctionType
ALU = mybir.AluOpType
EPS = 1e-5

BF = ml_dtypes.bfloat16


def _cfg(B, L, DM, DI, NST, DTR, DCONV, V, NC, LC, rs_f32=True):
    assert (B * L) % NC == 0 and DI % NC == 0
    c = dict(B=B, L=L, DM=DM, DI=DI, NST=NST, DTR=DTR, DCONV=DCONV, V=V,
             NC=NC, LC=LC, rs_f32=rs_f32)
    c["DSH"] = DI // NC            # channels per core
    c["TSH"] = (B * L) // NC       # tokens per core (preamble/in_proj shard)
    c["TT"] = c["TSH"] // 128      # token tiles per core
    c["DT"] = c["DSH"] // 128      # channel tiles per core
    c["KT"] = DM // 128            # d_model k-tiles
    c["OMT"] = DM // 128           # out_proj m tiles
    c["R2"] = DTR + 2 * NST
    c["NCH"] = B * (L // LC)       # total scan chunks
    c["NRS"] = c["NCH"]            # one reduce-scatter group per chunk
    c["TPG"] = LC                  # tokens per group
    c["MSH"] = DM // NC
    c["HN"] = NST // 2             # states per half
    c["LCP"] = LC + 1              # chunk + carry column
    assert c["TSH"] % 128 == 0 and c["DSH"] % 128 == 0 and L % LC == 0
    assert DTR == 128 and LC % 512 == 0
    return c


CFG = _cfg(B=2, L=2048, DM=2048, DI=4096, NST=16, DTR=128, DCONV=4, V=50257,
           NC=8, LC=512, rs_f32=True)


def build_nc(c, num_devices=None, reps=1):
    NC = c["NC"]
    B, L, DM, DI = c["B"], c["L"], c["DM"], c["DI"]
    NST, DTR, DCONV, V = c["NST"], c["DTR"], c["DCONV"], c["V"]
    DSH, TSH, TT, DT = c["DSH"], c["TSH"], c["TT"], c["DT"]
    KT, OMT, R2 = c["KT"], c["OMT"], c["R2"]
    LC, LCP, HN = c["LC"], c["LCP"], c["HN"]
    NCH, NRS, TPG, MSH = c["NCH"], c["NRS"], c["TPG"], c["MSH"]
    BL = B * L
    NCHB = L // LC                # chunks per batch entry
    MT = 2 * DI // 128            # in_proj m tiles (x then z)
    MTX = DI // 128
    RSDT = F32 if c["rs_f32"] else BF16
    groups = [list(range(NC))]
    SRB = TSH                     # tokens per a2a source block

    nc = bacc.Bacc("TRN2", target_bir_lowering=False, debug=False,
                   num_devices=num_devices or NC)

    # ---- kernel I/O ----
    ids_t = nc.dram_tensor("ids", [128, TT], I32, kind="ExternalInput")
    resid_t = nc.dram_tensor("resid", [TSH, DM], F32, kind="ExternalInput")
    embed_t = nc.dram_tensor("embed", [V, DM], F32, kind="ExternalInput")
    w_in_t = nc.dram_tensor("w_in", [DM, 2 * DI], BF16, kind="ExternalInput")
    convw_t = nc.dram_tensor("convw", [128, DT * DCONV], F32, kind="ExternalInput")
    convb_t = nc.dram_tensor("convb", [128, DT], F32, kind="ExternalInput")
    xpw_t = nc.dram_tensor("xpw", [DSH, R2], BF16, kind="ExternalInput")
    dtw_t = nc.dram_tensor("dtw", [DTR, DSH], BF16, kind="ExternalInput")
    dtb_t = nc.dram_tensor("dtb", [128, DT], F32, kind="ExternalInput")
    A_t = nc.dram_tensor("A", [128, DT * NST], F32, kind="ExternalInput")
    Dp_t = nc.dram_tensor("Dp", [128, DT], F32, kind="ExternalInput")
    wo_t = nc.dram_tensor("wo", [DSH, DM], BF16, kind="ExternalInput")

    resid_out_t = nc.dram_tensor("resid_out", [TSH, DM], F32, kind="ExternalOutput")
    y_out_t = nc.dram_tensor("y_out", [NRS, MSH, TPG], F32, kind="ExternalOutput")

    with tile.TileContext(nc) as tc:
        with (
            tc.tile_pool(name="dram", bufs=1, space="DRAM") as dram,
            tc.tile_pool(name="const", bufs=1) as const,
        ):
            # ---- constants to SBUF ----
            ids_sb = const.tile([128, TT], I32)
            nc.sync.dma_start(ids_sb[:], ids_t[:])
            convw_sb = const.tile([128, DT * DCONV], F32)
            nc.sync.dma_start(convw_sb[:], convw_t[:])
            convb_sb = const.tile([128, DT], F32)
            nc.sync.dma_start(convb_sb[:], convb_t[:])
            dtb_sb = const.tile([128, DT], F32)
            nc.sync.dma_start(dtb_sb[:], dtb_t[:])
            A_sb = const.tile([128, DT * NST], F32)
            nc.sync.dma_start(A_sb[:], A_t[:])
            Dp_sb = const.tile([128, DT], F32)
            nc.sync.dma_start(Dp_sb[:], Dp_t[:])
            dtw_sb = const.tile([DTR, DSH], BF16)
            nc.sync.dma_start(dtw_sb[:], dtw_t[:])
            xpw_sb = const.tile([128, DT, R2], BF16)
            nc.sync.dma_start(xpw_sb[:], xpw_t[:].rearrange("(k p) r -> p k r", p=128))
            wo_sb = const.tile([128, DT, DM], BF16)
            nc.sync.dma_start(wo_sb[:], wo_t[:].rearrange("(k p) m -> p k m", p=128))
            ident = const.tile([128, 128], BF16)
            make_identity(nc, ident[:])
            zero_b = const.tile([128, 1], F32)
            nc.vector.memset(zero_b[:], 0.0)
            eps_b = const.tile([128, 1], F32)
            nc.vector.memset(eps_b[:], EPS)
            # conv diag weights: cdiag[p, dti, j, q] = delta(p,q)*convw[p, dti*DCONV+j]
            cdiag = const.tile([128, DT, DCONV, 128], BF16)
            for dti in range(DT):
                for j in range(DCONV):
                    nc.vector.tensor_scalar(
                        cdiag[:, dti, j, :], ident[:],
                        convw_sb[:, dti * DCONV + j:dti * DCONV + j + 1],
                        None, ALU.mult)

            for _rep in range(reps):
                # ---- internal DRAM ----
                x_loc = dram.tile([DI, TSH], BF16, tag=f"x_loc{_rep}", name=f"x_loc{_rep}")
                z_loc = dram.tile([DI, TSH], BF16, tag=f"z_loc{_rep}", name=f"z_loc{_rep}")
                x_a2a = dram.tile([NC, DSH, SRB], BF16, tag=f"x_a2a{_rep}", name=f"x_a2a{_rep}")
                z_a2a = dram.tile([NC, DSH, SRB], BF16, tag=f"z_a2a{_rep}", name=f"z_a2a{_rep}")
                xdmaj = dram.tile([DSH, BL], BF16, tag=f"xdmaj{_rep}", name=f"xdmaj{_rep}")
                xdbl_par = [dram.tile([R2, L], F32, tag=f"xdp{b}_{_rep}", name=f"xdp{b}_{_rep}")
                            for b in range(B)]
                xdbl = [dram.tile([R2, L], F32, addr_space="Shared", tag=f"xd{b}_{_rep}",
                                  name=f"xd{b}_{_rep}") for b in range(B)]
                bc16 = [dram.tile([2 * NST, L], BF16, tag=f"bc16_{b}_{_rep}",
                                  name=f"bc16_{b}_{_rep}") for b in range(B)]
                op_par = [dram.tile([DM, TPG], RSDT, tag=f"opp{g}_{_rep}", name=f"opp{g}_{_rep}")
                          for g in range(NRS)]
                rs_out = [dram.tile([MSH, TPG], RSDT, tag=f"rso{g}_{_rep}",
                                   name=f"rso{g}_{_rep}") for g in range(NRS)]

                # ========== preamble: gather + residual + rmsnorm ==========
                with (
                    tc.tile_pool(name="hst", bufs=1) as hstp,
                    tc.tile_pool(name="pre", bufs=3) as pre,
                    tc.tile_pool(name="pre_ps", bufs=2, space="PSUM") as pre_ps,
                ):
                    hst = hstp.tile([128, KT, TSH], BF16)
                    for j in range(TT):
                        emb = pre.tile([128, DM], F32, tag="emb")
                        nc.gpsimd.indirect_dma_start(
                            out=emb[:], out_offset=None, in_=embed_t[:],
                            in_offset=IndirectOffsetOnAxis(ap=ids_sb[:, j:j + 1], axis=0),
                        )
                        res = pre.tile([128, DM], F32, tag="res")
                        nc.sync.dma_start(res[:], resid_t[j * 128:(j + 1) * 128, :])
                        radd = pre.tile([128, DM], F32, tag="radd")
                        nc.vector.tensor_add(radd[:], emb[:], res[:])
                        nc.sync.dma_start(resid_out_t[j * 128:(j + 1) * 128, :], radd[:])
                        # rms scale = 1/sqrt(mean(x^2) + eps)
                        sq = pre.tile([128, DM], F32, tag="sq")
                        ss = pre.tile([128, 1], F32, tag="ss")
                        nc.scalar.activation(sq[:], radd[:], AF.Square, bias=zero_b[:, 0:1],
                                             accum_out=ss[:])
                        rr = pre.tile([128, 1], F32, tag="rr")
                        nc.scalar.activation(rr[:], ss[:], AF.Sqrt, bias=eps_b[:, 0:1],
                                             scale=1.0 / DM)
                        inv = pre.tile([128, 1], F32, tag="inv")
                        nc.vector.reciprocal(inv[:], rr[:])
                        hsb = pre.tile([128, DM], BF16, tag="hsb")
                        nc.vector.tensor_scalar_mul(hsb[:], radd[:], inv[:, 0:1])
                        # transpose to d-major into persistent hst
                        for dcol in range(KT):
                            pt = pre_ps.tile([128, 128], BF16, tag="pt")
                            nc.tensor.transpose(pt[:], hsb[:, dcol * 128:(dcol + 1) * 128],
                                                ident[:])
                            nc.vector.tensor_copy(hst[:, dcol, j * 128:(j + 1) * 128], pt[:])

                    # ========== in_proj (token-major, streamed weights) =====
                    with (
                        tc.tile_pool(name="wA", bufs=3) as wA,
                        tc.tile_pool(name="psA", bufs=4, space="PSUM") as psA,
                        tc.tile_pool(name="xzA", bufs=3) as xzA,
                    ):
                        for m in range(MT):
                            wmt = wA.tile([128, KT, 128], BF16, tag="wmt")
                            nc.sync.dma_start(
                                wmt[:], w_in_t[:, m * 128:(m + 1) * 128]
                                .rearrange("(k p) e -> p k e", p=128))
                            ps = psA.tile([128, TSH], F32, tag="ps")
                            for k in range(KT):
                                nc.tensor.matmul(ps[:], lhsT=wmt[:, k, :],
                                                 rhs=hst[:, k, :], start=(k == 0),
                                                 stop=(k == KT - 1))
                            xm = xzA.tile([128, TSH], BF16, tag="xm")
                            if m < MTX:
                                nc.scalar.copy(xm[:], ps[:])
                                nc.sync.dma_start(x_loc[m * 128:(m + 1) * 128, :], xm[:])
                            else:
                                nc.scalar.activation(xm[:], ps[:], AF.Silu,
                                                     bias=zero_b[:, 0:1])
                                nc.sync.dma_start(
                                    z_loc[(m - MTX) * 128:(m - MTX + 1) * 128, :], xm[:])
                            if m == MTX - 1:
                                nc.gpsimd.collective_compute(
                                    "AllToAll", ALU.bypass, replica_groups=groups,
                                    ins=[x_loc[:].opt()], outs=[x_a2a[:].opt()],
                                )
                        nc.gpsimd.collective_compute(
                            "AllToAll", ALU.bypass, replica_groups=groups,
                            ins=[z_loc[:].opt()], outs=[z_a2a[:].opt()],
                        )

                # ========== causal depthwise conv1d (PE) + silu ==========
                with (
                    tc.tile_pool(name="cv", bufs=2) as cv,
                    tc.tile_pool(name="cvo", bufs=3) as cvo,
                    tc.tile_pool(name="psC", bufs=3, space="PSUM") as psC,
                ):
                    NCB = L // 512
                    for b in range(B):
                        for dti in range(DT):
                            xcp = cv.tile([128, DCONV - 1 + L], BF16, tag="xcp")
                            nc.vector.memset(xcp[:, 0:DCONV - 1], 0.0)
                            nc.sync.dma_start(
                                xcp[:, DCONV - 1:]
                                .rearrange("p (s t) -> p s t", s=NC // B),
                                x_a2a[b * (NC // B):(b + 1) * (NC // B),
                                      dti * 128:(dti + 1) * 128, :]
                                .rearrange("s p t -> p s t"))
                            for cb in range(NCB):
                                psc = psC.tile([128, 512], F32, tag="psc")
                                for j in range(DCONV):
                                    nc.tensor.matmul(
                                        psc[:], lhsT=cdiag[:, dti, j, :],
                                        rhs=xcp[:, cb * 512 + j:cb * 512 + j + 512],
                                        start=(j == 0), stop=(j == DCONV - 1))
                                xs = cvo.tile([128, 512], BF16, tag="xs")
                                nc.scalar.activation(xs[:], psc[:], AF.Silu,
                                                     bias=convb_sb[:, dti:dti + 1])
                                nc.sync.dma_start(
                                    xdmaj[dti * 128:(dti + 1) * 128,
                                          b * L + cb * 512:b * L + (cb + 1) * 512],
                                    xs[:])

                # ========== x_proj partials + allreduce ==========
                with (
                    tc.tile_pool(name="xp", bufs=2) as xp,
                    tc.tile_pool(name="psX", bufs=2, space="PSUM") as psX,
                    tc.tile_pool(name="xpo", bufs=3) as xpo,
                ):
                    NBT = L // TSH
                    for b in range(B):
                        for tb in range(NBT):
                            col0 = b * L + tb * TSH
                            xt = xp.tile([128, DT, TSH], BF16, tag="xt")
                            nc.sync.dma_start(
                                xt[:], xdmaj[:, col0:col0 + TSH]
                                .rearrange("(k p) t -> p k t", p=128))
                            ps1 = psX.tile([DTR, TSH], F32, tag="ps1")
                            ps2 = psX.tile([2 * NST, TSH], F32, tag="ps2")
                            for k in range(DT):
                                nc.tensor.matmul(ps1[:], lhsT=xpw_sb[:, k, 0:DTR],
                                                 rhs=xt[:, k, :], start=(k == 0),
                                                 stop=(k == DT - 1))
                            for k in range(DT):
                                nc.tensor.matmul(ps2[:], lhsT=xpw_sb[:, k, DTR:R2],
                                                 rhs=xt[:, k, :], start=(k == 0),
                                                 stop=(k == DT - 1))
                            s1 = xpo.tile([DTR, TSH], F32, tag="s1")
                            nc.scalar.copy(s1[:], ps1[:])
                            s2 = xpo.tile([2 * NST, TSH], F32, tag="s2")
                            nc.scalar.copy(s2[:], ps2[:])
                            nc.sync.dma_start(
                                xdbl_par[b][0:DTR, tb * TSH:(tb + 1) * TSH], s1[:])
                            nc.sync.dma_start(
                                xdbl_par[b][DTR:R2, tb * TSH:(tb + 1) * TSH], s2[:])
                        nc.gpsimd.collective_compute(
                            "AllReduce", ALU.add, replica_groups=groups,
                            ins=[xdbl_par[b][:].opt()], outs=[xdbl[b][:].opt()],
                        )

                # ========== scan + out_proj ==========
                with (
                    tc.tile_pool(name="dtf", bufs=1) as dtfp,
                    tc.tile_pool(name="bcc", bufs=1) as bcc,
                    tc.tile_pool(name="sc_in", bufs=2) as scin,
                    tc.tile_pool(name="dAp", bufs=2) as dAp,
                    tc.tile_pool(name="uBp", bufs=2) as uBp,
                    tc.tile_pool(name="htp", bufs=2) as htp,
                    tc.tile_pool(name="trp", bufs=2) as trp,
                    tc.tile_pool(name="yp", bufs=DT + 2) as yp,
                    tc.tile_pool(name="cry", bufs=1) as cry,
                    tc.tile_pool(name="psO", bufs=4, space="PSUM") as psO,
                ):
                    carry = cry.tile([128, DT, NST], BF16)
                    for b in range(B):
                        # ---- dt_proj + softplus for the whole batch entry ----
                        dtf = dtfp.tile([128, DT, L], BF16, tag="dtf")
                        with (
                            tc.tile_pool(name="dtt", bufs=1) as dtt,
                            tc.tile_pool(name="psD", bufs=2, space="PSUM") as psD,
                        ):
                            dtr = dtt.tile([DTR, L], F32, tag="dtr")
                            nc.sync.dma_start(dtr[:], xdbl[b][0:DTR, :])
                            dtr16 = dtt.tile([DTR, L], BF16, tag="dtr16")
                            nc.vector.tensor_copy(dtr16[:], dtr[:])
                            # B/C rows -> bf16 in DRAM (for replicated DMA loads)
                            bcf = dtt.tile([2 * NST, L], F32, tag="bcf")
                            nc.sync.dma_start(bcf[:], xdbl[b][DTR:R2, :])
                            bcb = dtt.tile([2 * NST, L], BF16, tag="bcb")
                            nc.vector.tensor_copy(bcb[:], bcf[:])
                            nc.sync.dma_start(bc16[b][:], bcb[:])
                            for dti in range(DT):
                                for hh in range(L // 512):
                                    psd = psD.tile([128, 512], F32, tag="psd")
                                    nc.tensor.matmul(
                                        psd[:],
                                        lhsT=dtw_sb[:, dti * 128:(dti + 1) * 128],
                                        rhs=dtr16[:, hh * 512:(hh + 1) * 512],
                                        start=True, stop=True)
                                    nc.scalar.activation(
                                        dtf[:, dti, hh * 512:(hh + 1) * 512], psd[:],
                                        AF.Exp, bias=dtb_sb[:, dti:dti + 1])
                            # softplus = ln(1 + exp(.)) — one big in-place Ln
                            nc.scalar.activation(
                                dtf[:].rearrange("p k l -> p (k l)"),
                                dtf[:].rearrange("p k l -> p (k l)"),
                                AF.Ln, bias=1.0)
                        nc.vector.memset(carry[:], 0.0)
                        for ci in range(NCHB):
                            lc0 = ci * LC
                            gc = b * NCHB + ci      # global chunk == RS group
                            # ---- B/C broadcast via replicated DMA ----
                            bbc = bcc.tile([128, NST, LC], BF16, tag="bbc")
                            nc.sync.dma_start(
                                bbc[:], bc16[b][0:NST, lc0:lc0 + LC]
                                .rearrange("n l -> () n l").to_broadcast([128, NST, LC]))
                            cbc = bcc.tile([128, NST, LC], BF16, tag="cbc")
                            nc.sync.dma_start(
                                cbc[:], bc16[b][NST:2 * NST, lc0:lc0 + LC]
                                .rearrange("n l -> () n l").to_broadcast([128, NST, LC]))
                            xt4 = scin.tile([128, DT, LC], BF16, tag="xt4")
                            nc.sync.dma_start(
                                xt4[:], xdmaj[:, b * L + lc0:b * L + lc0 + LC]
                                .rearrange("(k p) l -> p k l", p=128))
                            # z tokens for this chunk live in a2a source blocks
                            zs4 = scin.tile([128, DT, LC], BF16, tag="zs4")
                            s0 = (b * L + lc0) // SRB
                            ns = max(1, LC // SRB)
                            toff = (b * L + lc0) % SRB
                            nc.sync.dma_start(
                                zs4[:].rearrange("p k (s t) -> p k s t", s=ns),
                                z_a2a[s0:s0 + ns, :, toff:toff + min(LC, SRB)]
                                .rearrange("s (k p) t -> p k s t", p=128))
                            yf_tiles = []
                            for dti in range(DT):
                                dslc = dtf[:, dti, lc0:lc0 + LC]
                                xts = xt4[:, dti, :]
                                ut = scin.tile([128, LC], BF16, tag="ut")
                                nc.vector.tensor_tensor(ut[:], dslc, xts, ALU.mult)
                                yr = None
                                for h in range(2):
                                    n0 = h * HN
                                    dA = dAp.tile([128, HN, LCP], BF16, tag="dA")
                                    nc.vector.memset(dA[:, :, 0:1], 0.0)
                                    for n in range(HN):
                                        col = dti * NST + n0 + n
                                        nc.scalar.activation(
                                            dA[:, n, 1:], dslc, AF.Exp,
                                            bias=zero_b[:, 0:1],
                                            scale=A_sb[:, col:col + 1])
                                    uB = uBp.tile([128, HN, LCP], BF16, tag="uB")
                                    if ci == 0:
                                        nc.vector.memset(uB[:, :, 0:1], 0.0)
                                    else:
                                        nc.vector.tensor_copy(
                                            uB[:, :, 0:1],
                                            carry[:, dti, n0:n0 + HN]
                                            .rearrange("p n -> p n ()"))
                                    u3 = ut[:].rearrange("p (o l) -> p o l", o=1) \
                                              .to_broadcast([128, HN, LC])
                                    nc.vector.tensor_tensor(
                                        uB[:, :, 1:], u3, bbc[:, n0:n0 + HN, :],
                                        ALU.mult)
                                    ht = htp.tile([128, HN, LCP], BF16, tag="ht")
                                    nc.vector.tensor_tensor_scan(
                                        ht[:].rearrange("p n l -> p (n l)"),
                                        dA[:].rearrange("p n l -> p (n l)"),
                                        uB[:].rearrange("p n l -> p (n l)"),
                                        0.0, ALU.mult, ALU.add)
                                    nc.vector.tensor_copy(
                                        carry[:, dti, n0:n0 + HN],
                                        ht[:, :, LC:LCP].rearrange("p n o -> p (n o)"))
                                    ym = uBp.tile([128, HN, LC], BF16, tag="ym")
                                    nc.vector.tensor_tensor(
                                        ym[:], ht[:, :, 1:], cbc[:, n0:n0 + HN, :],
                                        ALU.mult)
                                    t4 = trp.tile([128, HN // 2, LC], BF16, tag="t4")
                                    nc.vector.tensor_tensor(
                                        t4[:], ym[:, 0:HN // 2, :], ym[:, HN // 2:HN, :],
                                        ALU.add)
                                    t2 = trp.tile([128, HN // 4, LC], BF16, tag="t2")
                                    nc.vector.tensor_tensor(
                                        t2[:], t4[:, 0:HN // 4, :], t4[:, HN // 4:HN // 2, :],
                                        ALU.add)
                                    t1 = trp.tile([128, LC], BF16, tag="t1")
                                    nc.vector.tensor_tensor(
                                        t1[:], t2[:, 0, :], t2[:, 1, :], ALU.add)
                                    if h == 0:
                                        yr = t1
                                    else:
                                        yh = trp.tile([128, LC], BF16, tag="yh")
                                        nc.vector.tensor_tensor(yh[:], yr[:], t1[:],
                                                                ALU.add)
                                        yr = yh
                                # y += u*D ; y *= silu(z)
                                yD = trp.tile([128, LC], BF16, tag="yD")
                                nc.vector.scalar_tensor_tensor(
                                    yD[:], xts, Dp_sb[:, dti:dti + 1], yr[:],
                                    ALU.mult, ALU.add)
                                yf = yp.tile([128, LC], BF16, tag="yf")
                                nc.vector.tensor_tensor(yf[:], yD[:], zs4[:, dti, :],
                                                        ALU.mult)
                                yf_tiles.append(yf)
                            # ---- out_proj partial straight from PSUM to HBM ----
                            with tc.tile_pool(name=f"ob{gc}", bufs=4) as obp:
                                for m in range(OMT):
                                    for colh in range(LC // 512):
                                        pso = psO.tile([128, 512], F32, tag="pso")
                                        for k in range(DT):
                                            nc.tensor.matmul(
                                                pso[:],
                                                lhsT=wo_sb[:, k, m * 128:(m + 1) * 128],
                                                rhs=yf_tiles[k][:, colh * 512:(colh + 1) * 512],
                                                start=(k == 0), stop=(k == DT - 1))
                                        ob = obp.tile([128, 512], RSDT, tag="ob")
                                        nc.scalar.copy(ob[:], pso[:])
                                        nc.sync.dma_start(
                                            op_par[gc][m * 128:(m + 1) * 128,
                                                       colh * 512:(colh + 1) * 512],
                                            ob[:])
                            # ---- reduce-scatter this chunk ----
                            nc.gpsimd.collective_compute(
                                "ReduceScatter", ALU.add, replica_groups=groups,
                                ins=[op_par[gc][:].opt()], outs=[rs_out[gc][:].opt()],
                            )
                            nc.sync.dma_start(
                                y_out_t[gc:gc + 1],
                                rs_out[gc][:].rearrange("m t -> (m t)")
                                .rearrange("(o m t) -> o m t", o=1, m=MSH))
    nc.compile()
    return nc


# ===================== host-side sharding =====================

def make_in_maps(c, inputs):
    NC, DSH, TSH, DT = c["NC"], c["DSH"], c["TSH"], c["DT"]
    B, L, DM, DI = c["B"], c["L"], c["DM"], c["DI"]
    NST, DTR, DCONV, V = c["NST"], c["DTR"], c["DCONV"], c["V"]

    ids = np.asarray(inputs["input_ids"]).reshape(-1).astype(np.int32)
    resid = np.asarray(inputs["residual"], np.float32).reshape(B * L, DM)
    embed = np.ascontiguousarray(np.asarray(inputs["embed"], np.float32))
    norm_w = np.asarray(inputs["norm_w"], np.float32)
    w_in = np.asarray(inputs["in_proj_w"], np.float32) * norm_w[None, :]
    w_full = np.ascontiguousarray(w_in.T).astype(BF)  # (DM, 2*DI)
    conv_w = np.asarray(inputs["conv_w"], np.float32)
    conv_b = np.asarray(inputs["conv_b"], np.float32)
    xpw = np.asarray(inputs["x_proj_w"], np.float32)
    dtw = np.asarray(inputs["dt_proj_w"], np.float32)
    dtb = np.asarray(inputs["dt_proj_b"], np.float32)
    A = (-np.exp(np.asarray(inputs["A_log"], np.float32))).astype(np.float32)
    Dp = np.asarray(inputs["D_param"], np.float32)
    wo = np.asarray(inputs["out_proj_w"], np.float32)

    in_maps = []
    for cc in range(NC):
        ch = slice(cc * DSH, (cc + 1) * DSH)
        cw = conv_w[ch].reshape(DT, 128, DCONV).transpose(1, 0, 2).reshape(128, DT * DCONV)
        cb = conv_b[ch].reshape(DT, 128).T
        dtb_c = dtb[ch].reshape(DT, 128).T
        A_c = A[ch].reshape(DT, 128, NST).transpose(1, 0, 2).reshape(128, DT * NST)
        Dp_c = Dp[ch].reshape(DT, 128).T
        in_maps.append({
            "ids": ids[cc * TSH:(cc + 1) * TSH].reshape(-1, 128).T.copy(),
            "resid": resid[cc * TSH:(cc + 1) * TSH].copy(),
            "embed": embed,
            "w_in": w_full,
            "convw": np.ascontiguousarray(cw),
            "convb": np.ascontiguousarray(cb),
            "xpw": np.ascontiguousarray(xpw[:, ch].T).astype(BF),
            "dtw": np.ascontiguousarray(dtw[ch, :].T).astype(BF),
            "dtb": np.ascontiguousarray(dtb_c),
            "A": np.ascontiguousarray(A_c),
            "Dp": np.ascontiguousarray(Dp_c),
            "wo": np.ascontiguousarray(wo[:, ch].T).astype(BF),
        })
    return in_maps


def assemble(c, results):
    NC, TSH, DM, B, L = c["NC"], c["TSH"], c["DM"], c["B"], c["L"]
    NRS, TPG, MSH = c["NRS"], c["TPG"], c["MSH"]
    resid = np.concatenate([results[cc]["resid_out"] for cc in range(NC)], 0)
    y = np.stack([results[cc]["y_out"] for cc in range(NC)], 0)  # (NC,NRS,MSH,TPG)
    hs = y.transpose(1, 3, 0, 2).reshape(B * L, DM)
    return (hs.reshape(B, L, DM).astype(np.float32),
            resid.reshape(B, L, DM).astype(np.float32))


_COMPILED = {}


def get_compiled(c=None):
    key = id(c) if c is not None else "default"
    if key not in _COMPILED:
        _COMPILED[key] = build_nc(c or CFG)
    return _COMPILED[key]


def get_compiled_replicated(reps, c=None):
    key = ("rep", reps, id(c) if c is not None else "default")
    if key not in _COMPILED:
        _COMPILED[key] = build_nc(c or CFG, reps=reps)
    return _COMPILED[key], reps


def kernel(**inputs):
    c = CFG
    nc = get_compiled(c)
    in_maps = make_in_maps(c, inputs)
    res = run_bass_kernel_spmd(nc, in_maps, core_ids=list(range(c["NC"])))
    return assemble(c, res.results)


# revision 15
# speedup vs baseline: 1.5237x; 1.5237x over previous
"""Trainium2 Bass kernel for a Mamba block (embed lookup -> residual add ->
RMSNorm -> Mamba(in_proj, causal conv1d, selective scan, out_proj)).

v2 design:
- preamble (embed gather + residual + RMSNorm) token-sharded; hs kept in SBUF
  d-major (PE transposes), no AllGather.
- in_proj computed token-major with the FULL weight streamed from HBM; the
  xz activations are resharded to d-major via AllToAll (x and z separately).
- causal depthwise conv1d on the PE (per-channel diag-weight matmuls).
- x_proj partials AllReduced per batch entry.
- selective scan: chained tensor_tensor_scan over [8 states x (LC+1)] with
  carry-injection columns; dA via per-state Exp with per-partition A scale;
  softplus/silu as single fused activations; bf16 tree reduce over states.
- out_proj partials DMAd straight from PSUM to HBM; ReduceScatter per chunk.
"""

import numpy as np
import ml_dtypes

import concourse.bacc as bacc
import concourse.bass as bass
import concourse.mybir as mybir
import concourse.tile as tile
from concourse.bass import AP, IndirectOffsetOnAxis
from concourse.bass_utils import run_bass_kernel_spmd
from concourse.masks import make_identity

F32 = mybir.dt.float32
BF16 = mybir.dt.bfloat16
I32 = mybir.dt.int32
AF = mybir.ActivationFunctionType
ALU = mybir.AluOpType
EPS = 1e-5

BF = ml_dtypes.bfloat16


def _cfg(B, L, DM, DI, NST, DTR, DCONV, V, NC, LC, rs_f32=True):
    assert (B * L) % NC == 0 and DI % NC == 0
    c = dict(B=B, L=L, DM=DM, DI=DI, NST=NST, DTR=DTR, DCONV=DCONV, V=V,
             NC=NC, LC=LC, rs_f32=rs_f32)
    c["DSH"] = DI // NC            # channels per core
    c["TSH"] = (B * L) // NC       # tokens per core (preamble/in_proj shard)
    c["TT"] = c["TSH"] // 128      # token tiles per core
    c["DT"] = c["DSH"] // 128      # channel tiles per core
    c["KT"] = DM // 128            # d_model k-tiles
    c["OMT"] = DM // 128           # out_proj m tiles
    c["R2"] = DTR + 2 * NST
    c["NCH"] = B * (L // LC)       # total scan chunks
    c["NRS"] = c["NCH"]            # one reduce-scatter group per chunk
    c["TPG"] = LC                  # tokens per group
    c["MSH"] = DM // NC
    c["HN"] = NST // 2             # states per half
    c["LCP"] = LC + 8              # chunk + 8 pad cols (carry in col 7, data 8..)
    assert c["TSH"] % 128 == 0 and c["DSH"] % 128 == 0 and L % LC == 0
    assert DTR == 128 and LC % 512 == 0
    return c


CFG = _cfg(B=2, L=2048, DM=2048, DI=4096, NST=16, DTR=128, DCONV=4, V=50257,
           NC=8, LC=512, rs_f32=True)


def build_nc(c, num_devices=None, reps=1):
    NC = c["NC"]
    B, L, DM, DI = c["B"], c["L"], c["DM"], c["DI"]
    NST, DTR, DCONV, V = c["NST"], c["DTR"], c["DCONV"], c["V"]
    DSH, TSH, TT, DT = c["DSH"], c["TSH"], c["TT"], c["DT"]
    KT, OMT, R2 = c["KT"], c["OMT"], c["R2"]
    LC, LCP, HN = c["LC"], c["LCP"], c["HN"]
    NCH, NRS, TPG, MSH = c["NCH"], c["NRS"], c["TPG"], c["MSH"]
    BL = B * L
    NCHB = L // LC                # chunks per batch entry
    MT = 2 * DI // 128            # in_proj m tiles (x then z)
    MTX = DI // 128
    RSDT = F32 if c["rs_f32"] else BF16
    groups = [list(range(NC))]
    SRB = TSH                     # tokens per a2a source block

    nc = bacc.Bacc("TRN2", target_bir_lowering=False, debug=False,
                   num_devices=num_devices or NC)

    # ---- kernel I/O ----
    ids_t = nc.dram_tensor("ids", [128, TT], I32, kind="ExternalInput")
    resid_t = nc.dram_tensor("resid", [TSH, DM], F32, kind="ExternalInput")
    embed_t = nc.dram_tensor("embed", [V, DM], F32, kind="ExternalInput")
    w_in_t = nc.dram_tensor("w_in", [DM, 2 * DI], BF16, kind="ExternalInput")
    convw_t = nc.dram_tensor("convw", [128, DT * DCONV], F32, kind="ExternalInput")
    convb_t = nc.dram_tensor("convb", [128, DT], F32, kind="ExternalInput")
    xpw_t = nc.dram_tensor("xpw", [DSH, R2], BF16, kind="ExternalInput")
    dtw_t = nc.dram_tensor("dtw", [DTR, DSH], BF16, kind="ExternalInput")
    dtb_t = nc.dram_tensor("dtb", [128, DT], F32, kind="ExternalInput")
    A_t = nc.dram_tensor("A", [128, DT * NST], F32, kind="ExternalInput")
    Dp_t = nc.dram_tensor("Dp", [128, DT], F32, kind="ExternalInput")
    wo_t = nc.dram_tensor("wo", [DSH, DM], BF16, kind="ExternalInput")

    resid_out_t = nc.dram_tensor("resid_out", [TSH, DM], F32, kind="ExternalOutput")
    y_out_t = nc.dram_tensor("y_out", [NRS, MSH, TPG], F32, kind="ExternalOutput")

    with tile.TileContext(nc) as tc:
        with (
            tc.tile_pool(name="dram", bufs=1, space="DRAM") as dram,
            tc.tile_pool(name="const", bufs=1) as const,
        ):
            # ---- constants to SBUF ----
            ids_sb = const.tile([128, TT], I32)
            nc.sync.dma_start(ids_sb[:], ids_t[:])
            convw_sb = const.tile([128, DT * DCONV], F32)
            nc.sync.dma_start(convw_sb[:], convw_t[:])
            convb_sb = const.tile([128, DT], F32)
            nc.sync.dma_start(convb_sb[:], convb_t[:])
            dtb_sb = const.tile([128, DT], F32)
            nc.sync.dma_start(dtb_sb[:], dtb_t[:])
            A_sb = const.tile([128, DT * NST], F32)
            nc.sync.dma_start(A_sb[:], A_t[:])
            Dp_sb = const.tile([128, DT], F32)
            nc.sync.dma_start(Dp_sb[:], Dp_t[:])
            dtw_sb = const.tile([DTR, DSH], BF16)
            nc.sync.dma_start(dtw_sb[:], dtw_t[:])
            xpw_sb = const.tile([128, DT, R2], BF16)
            nc.sync.dma_start(xpw_sb[:], xpw_t[:].rearrange("(k p) r -> p k r", p=128))
            wo_sb = const.tile([128, DT, DM], BF16)
            nc.sync.dma_start(wo_sb[:], wo_t[:].rearrange("(k p) m -> p k m", p=128))
            ident = const.tile([128, 128], BF16)
            make_identity(nc, ident[:])
            zero_b = const.tile([128, 1], F32)
            nc.vector.memset(zero_b[:], 0.0)
            eps_b = const.tile([128, 1], F32)
            nc.vector.memset(eps_b[:], EPS)
            # conv diag weights: cdiag[p, dti, j, q] = delta(p,q)*convw[p, dti*DCONV+j]
            cdiag = const.tile([128, DT, DCONV, 128], BF16)
            for dti in range(DT):
                for j in range(DCONV):
                    nc.vector.tensor_scalar(
                        cdiag[:, dti, j, :], ident[:],
                        convw_sb[:, dti * DCONV + j:dti * DCONV + j + 1],
                        None, ALU.mult)

            for _rep in range(reps):
                # ---- internal DRAM ----
                x_loc = dram.tile([DI, TSH], BF16, tag=f"x_loc{_rep}", name=f"x_loc{_rep}")
                z_loc = dram.tile([DI, TSH], BF16, tag=f"z_loc{_rep}", name=f"z_loc{_rep}")
                x_a2a = dram.tile([NC, DSH, SRB], BF16, tag=f"x_a2a{_rep}", name=f"x_a2a{_rep}")
                z_a2a = dram.tile([NC, DSH, SRB], BF16, tag=f"z_a2a{_rep}", name=f"z_a2a{_rep}")
                xdmaj = dram.tile([DSH, BL], BF16, tag=f"xdmaj{_rep}", name=f"xdmaj{_rep}")
                xdbl_par = [dram.tile([R2, L], F32, tag=f"xdp{b}_{_rep}", name=f"xdp{b}_{_rep}")
                            for b in range(B)]
                xdbl = [dram.tile([R2, L], F32, addr_space="Shared", tag=f"xd{b}_{_rep}",
                                  name=f"xd{b}_{_rep}") for b in range(B)]
                bc16 = [dram.tile([2 * NST, L], BF16, tag=f"bc16_{b}_{_rep}",
                                  name=f"bc16_{b}_{_rep}") for b in range(B)]
                op_par = [dram.tile([DM, TPG], RSDT, tag=f"opp{g}_{_rep}", name=f"opp{g}_{_rep}")
                          for g in range(NRS)]
                rs_out = [dram.tile([MSH, TPG], RSDT, tag=f"rso{g}_{_rep}",
                                   name=f"rso{g}_{_rep}") for g in range(NRS)]

                # ========== preamble: gather + residual + rmsnorm ==========
                with (
                    tc.tile_pool(name="hst", bufs=1) as hstp,
                    tc.tile_pool(name="pre", bufs=3) as pre,
                    tc.tile_pool(name="pre_ps", bufs=2, space="PSUM") as pre_ps,
                ):
                    hst = hstp.tile([128, KT, TSH], BF16)
                    for j in range(TT):
                        emb = pre.tile([128, DM], F32, tag="emb")
                        nc.gpsimd.indirect_dma_start(
                            out=emb[:], out_offset=None, in_=embed_t[:],
                            in_offset=IndirectOffsetOnAxis(ap=ids_sb[:, j:j + 1], axis=0),
                        )
                        res = pre.tile([128, DM], F32, tag="res")
                        nc.sync.dma_start(res[:], resid_t[j * 128:(j + 1) * 128, :])
                        radd = pre.tile([128, DM], F32, tag="radd")
                        nc.vector.tensor_add(radd[:], emb[:], res[:])
                        nc.sync.dma_start(resid_out_t[j * 128:(j + 1) * 128, :], radd[:])
                        # rms scale = 1/sqrt(mean(x^2) + eps)
                        sq = pre.tile([128, DM], F32, tag="sq")
                        ss = pre.tile([128, 1], F32, tag="ss")
                        nc.scalar.activation(sq[:], radd[:], AF.Square, bias=zero_b[:, 0:1],
                                             accum_out=ss[:])
                        rr = pre.tile([128, 1], F32, tag="rr")
                        nc.scalar.activation(rr[:], ss[:], AF.Sqrt, bias=eps_b[:, 0:1],
                                             scale=1.0 / DM)
                        inv = pre.tile([128, 1], F32, tag="inv")
                        nc.vector.reciprocal(inv[:], rr[:])
                        hsb = pre.tile([128, DM], BF16, tag="hsb")
                        nc.vector.tensor_scalar_mul(hsb[:], radd[:], inv[:, 0:1])
                        # transpose to d-major into persistent hst
                        for dcol in range(KT):
                            pt = pre_ps.tile([128, 128], BF16, tag="pt")
                            nc.tensor.transpose(pt[:], hsb[:, dcol * 128:(dcol + 1) * 128],
                                                ident[:])
                            nc.vector.tensor_copy(hst[:, dcol, j * 128:(j + 1) * 128], pt[:])

                    # ========== in_proj (token-major, streamed weights) =====
                    with (
                        tc.tile_pool(name="wA", bufs=3) as wA,
                        tc.tile_pool(name="psA", bufs=4, space="PSUM") as psA,
                        tc.tile_pool(name="xzA", bufs=3) as xzA,
                    ):
                        for m in range(MT):
                            wmt = wA.tile([128, KT, 128], BF16, tag="wmt")
                            nc.sync.dma_start(
                                wmt[:], w_in_t[:, m * 128:(m + 1) * 128]
                                .rearrange("(k p) e -> p k e", p=128))
                            ps = psA.tile([128, TSH], F32, tag="ps")
                            for k in range(KT):
                                nc.tensor.matmul(ps[:], lhsT=wmt[:, k, :],
                                                 rhs=hst[:, k, :], start=(k == 0),
                                                 stop=(k == KT - 1))
                            xm = xzA.tile([128, TSH], BF16, tag="xm")
                            if m < MTX:
                                nc.scalar.copy(xm[:], ps[:])
                                nc.sync.dma_start(x_loc[m * 128:(m + 1) * 128, :], xm[:])
                            else:
                                nc.scalar.activation(xm[:], ps[:], AF.Silu,
                                                     bias=zero_b[:, 0:1])
                                nc.sync.dma_start(
                                    z_loc[(m - MTX) * 128:(m - MTX + 1) * 128, :], xm[:])
                            if m == MTX - 1:
                                nc.gpsimd.collective_compute(
                                    "AllToAll", ALU.bypass, replica_groups=groups,
                                    ins=[x_loc[:].opt()], outs=[x_a2a[:].opt()],
                                )
                        nc.gpsimd.collective_compute(
                            "AllToAll", ALU.bypass, replica_groups=groups,
                            ins=[z_loc[:].opt()], outs=[z_a2a[:].opt()],
                        )

                # ========== causal depthwise conv1d (PE) + silu ==========
                with (
                    tc.tile_pool(name="cv", bufs=2) as cv,
                    tc.tile_pool(name="cvo", bufs=3) as cvo,
                    tc.tile_pool(name="psC", bufs=3, space="PSUM") as psC,
                ):
                    NCB = L // 512
                    for b in range(B):
                        for dti in range(DT):
                            xcp = cv.tile([128, DCONV - 1 + L], BF16, tag="xcp")
                            nc.vector.memset(xcp[:, 0:DCONV - 1], 0.0)
                            nc.sync.dma_start(
                                xcp[:, DCONV - 1:]
                                .rearrange("p (s t) -> p s t", s=NC // B),
                                x_a2a[b * (NC // B):(b + 1) * (NC // B),
                                      dti * 128:(dti + 1) * 128, :]
                                .rearrange("s p t -> p s t"))
                            for cb in range(NCB):
                                psc = psC.tile([128, 512], F32, tag="psc")
                                for j in range(DCONV):
                                    nc.tensor.matmul(
                                        psc[:], lhsT=cdiag[:, dti, j, :],
                                        rhs=xcp[:, cb * 512 + j:cb * 512 + j + 512],
                                        start=(j == 0), stop=(j == DCONV - 1))
                                xs = cvo.tile([128, 512], BF16, tag="xs")
                                nc.scalar.activation(xs[:], psc[:], AF.Silu,
                                                     bias=convb_sb[:, dti:dti + 1])
                                nc.sync.dma_start(
                                    xdmaj[dti * 128:(dti + 1) * 128,
                                          b * L + cb * 512:b * L + (cb + 1) * 512],
                                    xs[:])

                # ========== x_proj partials + allreduce ==========
                with (
                    tc.tile_pool(name="xp", bufs=2) as xp,
                    tc.tile_pool(name="psX", bufs=2, space="PSUM") as psX,
                    tc.tile_pool(name="xpo", bufs=3) as xpo,
                ):
                    NBT = L // TSH
                    for b in range(B):
                        for tb in range(NBT):
                            col0 = b * L + tb * TSH
                            xt = xp.tile([128, DT, TSH], BF16, tag="xt")
                            nc.sync.dma_start(
                                xt[:], xdmaj[:, col0:col0 + TSH]
                                .rearrange("(k p) t -> p k t", p=128))
                            ps1 = psX.tile([DTR, TSH], F32, tag="ps1")
                            ps2 = psX.tile([2 * NST, TSH], F32, tag="ps2")
                            for k in range(DT):
                                nc.tensor.matmul(ps1[:], lhsT=xpw_sb[:, k, 0:DTR],
                                                 rhs=xt[:, k, :], start=(k == 0),
                                                 stop=(k == DT - 1))
                            for k in range(DT):
                                nc.tensor.matmul(ps2[:], lhsT=xpw_sb[:, k, DTR:R2],
                                                 rhs=xt[:, k, :], start=(k == 0),
                                                 stop=(k == DT - 1))
                            s1 = xpo.tile([DTR, TSH], F32, tag="s1")
                            nc.scalar.copy(s1[:], ps1[:])
                            s2 = xpo.tile([2 * NST, TSH], F32, tag="s2")
                            nc.scalar.copy(s2[:], ps2[:])
                            nc.sync.dma_start(
                                xdbl_par[b][0:DTR, tb * TSH:(tb + 1) * TSH], s1[:])
                            nc.sync.dma_start(
                                xdbl_par[b][DTR:R2, tb * TSH:(tb + 1) * TSH], s2[:])
                        nc.gpsimd.collective_compute(
                            "AllReduce", ALU.add, replica_groups=groups,
                            ins=[xdbl_par[b][:].opt()], outs=[xdbl[b][:].opt()],
                        )

                # ========== scan + out_proj ==========
                with (
                    tc.tile_pool(name="dtf", bufs=1) as dtfp,
                    tc.tile_pool(name="bcc", bufs=2) as bcc,
                    tc.tile_pool(name="sc_in", bufs=2) as scin,
                    tc.tile_pool(name="dAp", bufs=2) as dAp,
                    tc.tile_pool(name="uBp", bufs=2) as uBp,
                    tc.tile_pool(name="htp", bufs=2) as htp,
                    tc.tile_pool(name="trp", bufs=2) as trp,
                    tc.tile_pool(name="yp", bufs=DT + 2) as yp,
                    tc.tile_pool(name="cry", bufs=1) as cry,
                    tc.tile_pool(name="psO", bufs=4, space="PSUM") as psO,
                ):
                    carry = cry.tile([128, DT, NST], BF16)
                    for b in range(B):
                        # ---- dt_proj + softplus for the whole batch entry ----
                        dtf = dtfp.tile([128, DT, L], BF16, tag="dtf")
                        with (
                            tc.tile_pool(name="dtt", bufs=1) as dtt,
                            tc.tile_pool(name="psD", bufs=2, space="PSUM") as psD,
                        ):
                            dtr = dtt.tile([DTR, L], F32, tag="dtr")
                            nc.sync.dma_start(dtr[:], xdbl[b][0:DTR, :])
                            dtr16 = dtt.tile([DTR, L], BF16, tag="dtr16")
                            nc.vector.tensor_copy(dtr16[:], dtr[:])
                            # B/C rows -> bf16 in DRAM (for replicated DMA loads)
                            bcf = dtt.tile([2 * NST, L], F32, tag="bcf")
                            nc.sync.dma_start(bcf[:], xdbl[b][DTR:R2, :])
                            bcb = dtt.tile([2 * NST, L], BF16, tag="bcb")
                            nc.vector.tensor_copy(bcb[:], bcf[:])
                            nc.sync.dma_start(bc16[b][:], bcb[:])
                            for dti in range(DT):
                                for hh in range(L // 512):
                                    psd = psD.tile([128, 512], F32, tag="psd")
                                    nc.tensor.matmul(
                                        psd[:],
                                        lhsT=dtw_sb[:, dti * 128:(dti + 1) * 128],
                                        rhs=dtr16[:, hh * 512:(hh + 1) * 512],
                                        start=True, stop=True)
                                    nc.scalar.activation(
                                        dtf[:, dti, hh * 512:(hh + 1) * 512], psd[:],
                                        AF.Exp, bias=dtb_sb[:, dti:dti + 1])
                            # softplus = ln(1 + exp(.)) — one big in-place Ln
                            nc.scalar.activation(
                                dtf[:].rearrange("p k l -> p (k l)"),
                                dtf[:].rearrange("p k l -> p (k l)"),
                                AF.Ln, bias=1.0)
                        nc.vector.memset(carry[:], 0.0)
                        for ci in range(NCHB):
                            lc0 = ci * LC
                            gc = b * NCHB + ci      # global chunk == RS group
                            # ---- B/C broadcast via replicated DMA (n-halves) ----
                            bbch, cbch = [], []
                            for h in range(2):
                                n0 = h * HN
                                bh = bcc.tile([128, HN, LC], BF16, tag="bbc")
                                nc.sync.dma_start(
                                    bh[:], bc16[b][n0:n0 + HN, lc0:lc0 + LC]
                                    .rearrange("n l -> () n l")
                                    .to_broadcast([128, HN, LC]))
                                bbch.append(bh)
                                chh = bcc.tile([128, HN, LC], BF16, tag="cbc")
                                nc.sync.dma_start(
                                    chh[:], bc16[b][NST + n0:NST + n0 + HN, lc0:lc0 + LC]
                                    .rearrange("n l -> () n l")
                                    .to_broadcast([128, HN, LC]))
                                cbch.append(chh)
                            xt4 = scin.tile([128, DT, LC], BF16, tag="xt4")
                            nc.sync.dma_start(
                                xt4[:], xdmaj[:, b * L + lc0:b * L + lc0 + LC]
                                .rearrange("(k p) l -> p k l", p=128))
                            # z tokens for this chunk live in a2a source blocks
                            zs4 = scin.tile([128, DT, LC], BF16, tag="zs4")
                            s0 = (b * L + lc0) // SRB
                            ns = max(1, LC // SRB)
                            toff = (b * L + lc0) % SRB
                            nc.sync.dma_start(
                                zs4[:].rearrange("p k (s t) -> p k s t", s=ns),
                                z_a2a[s0:s0 + ns, :, toff:toff + min(LC, SRB)]
                                .rearrange("s (k p) t -> p k s t", p=128))
                            yf_tiles = []
                            for dti in range(DT):
                                dslc = dtf[:, dti, lc0:lc0 + LC]
                                xts = xt4[:, dti, :]
                                ut = scin.tile([128, LC], BF16, tag="ut")
                                nc.vector.tensor_tensor(ut[:], dslc, xts, ALU.mult)
                                yr = None
                                for h in range(2):
                                    n0 = h * HN
                                    # pad cols 0..7: dA=0 resets the chain; carry
                                    # injected at col 7; data 16B-aligned at col 8.
                                    dA = dAp.tile([128, HN, LCP], BF16, tag="dA")
                                    nc.vector.memset(dA[:, :, 0:8], 0.0)
                                    for n in range(HN):
                                        col = dti * NST + n0 + n
                                        nc.scalar.activation(
                                            dA[:, n, 8:], dslc, AF.Exp,
                                            bias=zero_b[:, 0:1],
                                            scale=A_sb[:, col:col + 1])
                                    uB = uBp.tile([128, HN, LCP], BF16, tag="uB")
                                    nc.vector.memset(uB[:, :, 0:7], 0.0)
                                    if ci == 0:
                                        nc.vector.memset(uB[:, :, 7:8], 0.0)
                                    else:
                                        nc.vector.tensor_copy(
                                            uB[:, :, 7:8],
                                            carry[:, dti, n0:n0 + HN]
                                            .rearrange("p n -> p n ()"))
                                    u3 = ut[:].rearrange("p (o l) -> p o l", o=1) \
                                              .to_broadcast([128, HN, LC])
                                    nc.vector.tensor_tensor(
                                        uB[:, :, 8:], u3, bbch[h][:],
                                        ALU.mult)
                                    ht = htp.tile([128, HN, LCP], BF16, tag="ht")
                                    nc.vector.tensor_tensor_scan(
                                        ht[:].rearrange("p n l -> p (n l)"),
                                        dA[:].rearrange("p n l -> p (n l)"),
                                        uB[:].rearrange("p n l -> p (n l)"),
                                        0.0, ALU.mult, ALU.add)
                                    nc.vector.tensor_copy(
                                        carry[:, dti, n0:n0 + HN],
                                        ht[:, :, LCP - 1:LCP].rearrange("p n o -> p (n o)"))
                                    ym = uBp.tile([128, HN, LC], BF16, tag="ym")
                                    nc.vector.tensor_tensor(
                                        ym[:], ht[:, :, 8:], cbch[h][:],
                                        ALU.mult)
                                    t4 = trp.tile([128, HN // 2, LC], BF16, tag="t4")
                                    nc.vector.tensor_tensor(
                                        t4[:], ym[:, 0:HN // 2, :], ym[:, HN // 2:HN, :],
                                        ALU.add)
                                    t2 = trp.tile([128, HN // 4, LC], BF16, tag="t2")
                                    nc.vector.tensor_tensor(
                                        t2[:], t4[:, 0:HN // 4, :], t4[:, HN // 4:HN // 2, :],
                                        ALU.add)
                                    t1 = trp.tile([128, LC], BF16, tag="t1")
                                    nc.vector.tensor_tensor(
                                        t1[:], t2[:, 0, :], t2[:, 1, :], ALU.add)
                                    if h == 0:
                                        yr = t1
                                    else:
                                        yh = trp.tile([128, LC], BF16, tag="yh")
                                        nc.vector.tensor_tensor(yh[:], yr[:], t1[:],
                                                                ALU.add)
                                        yr = yh
                                # y += u*D ; y *= silu(z)
                                yD = trp.tile([128, LC], BF16, tag="yD")
                                nc.vector.scalar_tensor_tensor(
                                    yD[:], xts, Dp_sb[:, dti:dti + 1], yr[:],
                                    ALU.mult, ALU.add)
                                yf = yp.tile([128, LC], BF16, tag="yf")
                                nc.vector.tensor_tensor(yf[:], yD[:], zs4[:, dti, :],
                                                        ALU.mult)
                                yf_tiles.append(yf)
                            # ---- out_proj partial straight from PSUM to HBM ----
                            with tc.tile_pool(name=f"ob{gc}", bufs=4) as obp:
                                for m in range(OMT):
                                    for colh in range(LC // 512):
                                        pso = psO.tile([128, 512], F32, tag="pso")
                                        for k in range(DT):
                                            nc.tensor.matmul(
                                                pso[:],
                                                lhsT=wo_sb[:, k, m * 128:(m + 1) * 128],
                                                rhs=yf_tiles[k][:, colh * 512:(colh + 1) * 512],
                                                start=(k == 0), stop=(k == DT - 1))
                                        ob = obp.tile([128, 512], RSDT, tag="ob")
                                        nc.scalar.copy(ob[:], pso[:])
                                        nc.sync.dma_start(
                                            op_par[gc][m * 128:(m + 1) * 128,
                                                       colh * 512:(colh + 1) * 512],
                                            ob[:])
                            # ---- reduce-scatter this chunk ----
                            nc.gpsimd.collective_compute(
                                "ReduceScatter", ALU.add, replica_groups=groups,
                                ins=[op_par[gc][:].opt()], outs=[rs_out[gc][:].opt()],
                            )
                            nc.sync.dma_start(
                                y_out_t[gc:gc + 1],
                                rs_out[gc][:].rearrange("m t -> (m t)")
                                .rearrange("(o m t) -> o m t", o=1, m=MSH))
    nc.compile()
    return nc


# ===================== host-side sharding =====================

def make_in_maps(c, inputs):
    NC, DSH, TSH, DT = c["NC"], c["DSH"], c["TSH"], c["DT"]
    B, L, DM, DI = c["B"], c["L"], c["DM"], c["DI"]
    NST, DTR, DCONV, V = c["NST"], c["DTR"], c["DCONV"], c["V"]

    ids = np.asarray(inputs["input_ids"]).reshape(-1).astype(np.int32)
    resid = np.asarray(inputs["residual"], np.float32).reshape(B * L, DM)
    embed = np.ascontiguousarray(np.asarray(inputs["embed"], np.float32))
    norm_w = np.asarray(inputs["norm_w"], np.float32)
    w_in = np.asarray(inputs["in_proj_w"], np.float32) * norm_w[None, :]
    w_full = np.ascontiguousarray(w_in.T).astype(BF)  # (DM, 2*DI)
    conv_w = np.asarray(inputs["conv_w"], np.float32)
    conv_b = np.asarray(inputs["conv_b"], np.float32)
    xpw = np.asarray(inputs["x_proj_w"], np.float32)
    dtw = np.asarray(inputs["dt_proj_w"], np.float32)
    dtb = np.asarray(inputs["dt_proj_b"], np.float32)
    A = (-np.exp(np.asarray(inputs["A_log"], np.float32))).astype(np.float32)
    Dp = np.asarray(inputs["D_param"], np.float32)
    wo = np.asarray(inputs["out_proj_w"], np.float32)

    in_maps = []
    for cc in range(NC):
        ch = slice(cc * DSH, (cc + 1) * DSH)
        cw = conv_w[ch].reshape(DT, 128, DCONV).transpose(1, 0, 2).reshape(128, DT * DCONV)
        cb = conv_b[ch].reshape(DT, 128).T
        dtb_c = dtb[ch].reshape(DT, 128).T
        A_c = A[ch].reshape(DT, 128, NST).transpose(1, 0, 2).reshape(128, DT * NST)
        Dp_c = Dp[ch].reshape(DT, 128).T
        in_maps.append({
            "ids": ids[cc * TSH:(cc + 1) * TSH].reshape(-1, 128).T.copy(),
            "resid": resid[cc * TSH:(cc + 1) * TSH].copy(),
            "embed": embed,
            "w_in": w_full,
            "convw": np.ascontiguousarray(cw),
            "convb": np.ascontiguousarray(cb),
            "xpw": np.ascontiguousarray(xpw[:, ch].T).astype(BF),
            "dtw": np.ascontiguousarray(dtw[ch, :].T).astype(BF),
            "dtb": np.ascontiguousarray(dtb_c),
            "A": np.ascontiguousarray(A_c),
            "Dp": np.ascontiguousarray(Dp_c),
            "wo": np.ascontiguousarray(wo[:, ch].T).astype(BF),
        })
    return in_maps


def assemble(c, results):
    NC, TSH, DM, B, L = c["NC"], c["TSH"], c["DM"], c["B"], c["L"]
    NRS, TPG, MSH = c["NRS"], c["TPG"], c["MSH"]
    resid = np.concatenate([results[cc]["resid_out"] for cc in range(NC)], 0)
    y = np.stack([results[cc]["y_out"] for cc in range(NC)], 0)  # (NC,NRS,MSH,TPG)
    hs = y.transpose(1, 3, 0, 2).reshape(B * L, DM)
    return (hs.reshape(B, L, DM).astype(np.float32),
            resid.reshape(B, L, DM).astype(np.float32))


_COMPILED = {}


def get_compiled(c=None):
    key = id(c) if c is not None else "default"
    if key not in _COMPILED:
        _COMPILED[key] = build_nc(c or CFG)
    return _COMPILED[key]


def get_compiled_replicated(reps, c=None):
    key = ("rep", reps, id(c) if c is not None else "default")
    if key not in _COMPILED:
        _COMPILED[key] = build_nc(c or CFG, reps=reps)
    return _COMPILED[key], reps


def kernel(**inputs):
    c = CFG
    nc = get_compiled(c)
    in_maps = make_in_maps(c, inputs)
    res = run_bass_kernel_spmd(nc, in_maps, core_ids=list(range(c["NC"])))
    return assemble(c, res.results)
